# revision 21
# baseline (speedup 1.0000x reference)
"""Trainium2 Bass kernel for modulated conv1d (StyleGAN-style Conv1DMod).

Reference computation (per batch sample b):
  wm[k,c,f]  = kern[k,c,f] * coef * (style[b,c] + 1)        (modulate)
  denom[f]   = rsqrt(sum_{k,c} wm[k,c,f]^2)                 (demodulate)
  out[b,f,w] = denom[f] * sum_{k,c} wm[k,c,f] * feat[b,c,w+k-1]   (SAME conv)

Sharding: data-parallel over batch B=8 -> one sample per NeuronCore.

Schedule notes (v6):
 - All heavy I/O is bf16: feature and kern are cast to bf16 on the host
   (error budget: products are already bf16-rounded on the PE), and the
   output is stored bf16 and widened to f32 on the host.  HBM traffic
   drops 16.8 MB -> 8.9 MB per core, far below the ~41.5 us PE floor
   (192 matmuls x 216 ns), so the Tensor engine is the only roofline.
 - One resident x mega-tile [128, 2, 8194] holds the whole padded
   feature row (c = 2p + h); chunk boundaries need no halo duplication
   and conv taps are plain column slices.
 - DMA issues cost ~0.66 us of sequencer time each, so transfers are
   few and large, all on the Sync hardware queue, fully front-loaded:
   kern, then 5 feature pieces (first one small so matmuls start ~2.5us
   in).  style rides the GpSimd SWDGE so its 8 B descriptors stay off
   the fast queue.  Stores (one per chunk x f-tile) trail behind.
 - 3 dep-free dummy matmuls ramp the PE clock (HAM un-throttles after
   ~3.4 us of activity) without delaying the first real matmul.
 - denom matmuls sit between the i=1 and i=2 groups of chunk 0 so PSUM
   drains (DVE, demod scale fused, bf16 out) can start early.
"""

import numpy as np
import ml_dtypes

import concourse.bass as bass
import concourse.mybir as mybir
import concourse.tile as tile

B, C, W, K, F = 8, 256, 8192, 3, 256
COEF = 1.0 / float(np.sqrt(K * C))

P = 128
H = 2  # contraction groups: c = 2*p + h
FT = F // P  # 2 output-partition tiles
WCHUNK = 2048
NJ = W // WCHUNK  # 4 chunks
WTILE = 512  # matmul moving-operand width (PSUM bank = 512 f32)
NI = WCHUNK // WTILE  # 4 w-tiles per chunk
XW = W + 2  # mega tile cols: col t holds feat[t-1], cols 0 and W+1 are zero

MAX_WAITS = 1  # walrus codegen in this container rejects >1 sync wait per inst


def _split_sync_waits(nc, limit=MAX_WAITS):
    """Move excess sem-waits onto NoOps inserted before the offending
    instruction (same engine, program order preserved)."""
    uid = 0
    for fn in nc.m.functions:
        for bb in fn.blocks:
            insts = bb.instructions
            changed = False
            newlist = []
            for ins in insts:
                si = ins.sync_info
                if si is not None and len(si.on_wait) > limit:
                    waits = list(si.on_wait)
                    keep = waits[-limit:]
                    excess = waits[:-limit]
                    for k in range(0, len(excess), limit):
                        nop = mybir.InstNoOp(name=f"waitsplit-{uid}", ins=[], outs=[])
                        uid += 1
                        nop.engine = ins.engine
                        nop.sync_info = mybir.SyncInfo(
                            on_wait=excess[k : k + limit], on_update=[]
                        )
                        newlist.append(nop)
                    ins.sync_info = mybir.SyncInfo(
                        on_wait=keep, on_update=list(si.on_update)
                    )
                    changed = True
                newlist.append(ins)
            if changed:
                bb.instructions = newlist


def _conv1dmod_body(tc, feat, wk, out):
    nc = tc.nc
    f32 = mybir.dt.float32
    bf16 = mybir.dt.bfloat16
    add = mybir.AluOpType.add
    mult = mybir.AluOpType.mult

    featr = feat.rearrange("(p h) w -> p h w", h=H)

    with (
        tc.tile_pool(name="wbuf", bufs=1) as wbuf,
        tc.tile_pool(name="xbuf", bufs=1) as xbuf,
        tc.tile_pool(name="stage", bufs=4) as stage_pool,
        tc.tile_pool(name="psum", bufs=8, space="PSUM") as psum_pool,
    ):
        # ---- warm-up operand first on DVE: it gates the PE dummies ----
        wu_w = wbuf.tile([P, P], bf16, tag="wu_w")
        nc.vector.memset(wu_w[:], 0.0)
        warm = wbuf.tile([P, 1], f32, tag="warm")
        nc.vector.memset(warm[:], 1.0)
        ones = wbuf.tile([P, 1], bf16, tag="ones")
        nc.vector.memset(ones[:], 1.0)

        # ---- x mega-tile (bf16) + zero halo columns ----
        xm = xbuf.tile([P, H, XW], bf16, tag="xm")
        for h in range(H):
            nc.vector.memset(xm[:, h, 0:1], 0.0)
            nc.vector.memset(xm[:, h, XW - 1 : XW], 0.0)

        # ---- head DMAs.  wk = host-packed [kern | (1+style)*coef] with 3 KB
        # contiguous per-partition runs.  DMA completion semaphores lag the
        # last byte by 1-4us depending on concurrent HBM load, so only wk,
        # P0 and P1 move in the first window; P2-P4 issue on the Activation
        # ring behind the act-table load, keeping HBM quiet while the wk
        # receipt (which gates modulates -> the whole PE stream) returns.
        wkt = wbuf.tile([P, K * H * F + H], bf16, tag="wkt")
        nc.sync.dma_start(wkt[:], wk[:, :])
        pieces = [0, 518, 2048, 4096, 6144, 8192]
        a, b = pieces[0], pieces[1]
        nc.scalar.dma_start(xm[:, :, a + 1 : b + 1], featr[:, :, a:b])
        # Completion receipts serialize per HWDGE ring (~+2-3us per DMA), so
        # a ring's second semaphore can't beat the PE's i=1 demand (~2.6us
        # into the stream).  The i=1 half of P1 rides the independent GpSimd
        # SWDGE path instead; the rest of P1 queues behind wk on Sync.
        nc.gpsimd.dma_start(xm[:, :, 519:1032], featr[:, :, 518:1031])
        nc.sync.dma_start(xm[:, :, 1032:2049], featr[:, :, 1031:2048])

        # warm the Scalar activation table (Sqrt) off the critical path
        warm2 = wbuf.tile([P, 1], f32, tag="warm2")
        nc.scalar.sqrt(warm2[:], warm[:])

        for pc in range(2, len(pieces) - 1):
            a, b = pieces[pc], pieces[pc + 1]
            nc.scalar.dma_start(xm[:, :, a + 1 : b + 1], featr[:, :, a:b])

        # ---- PE p-state warm-up: dep-free dummy matmuls keep the Tensor
        # engine busy (HAM un-throttles to 2.4 GHz after ~3.4us of activity)
        # and bridge all the way to wm-ready (~4.3us), so the real stream
        # starts warm with no PE-idle window for the HAM to re-throttle in.
        # N=64 keeps the bridge quantization (and any overshoot) tiny.
        wu_ps = psum_pool.tile([P, P], f32, tag="ps")
        for _ in range(70):
            nc.tensor.matmul(
                wu_ps[:, 0:64], wu_w[:], wu_w[:, 0:64], start=True, stop=True
            )

        # ---- modulate (bf16 weights) on DVE; scale vector comes in
        # pre-folded as (1 + style) * coef from the host, bf16, packed as
        # the last two wk columns ----
        s1f = wbuf.tile([P, H], f32, tag="s1f")
        nc.vector.tensor_scalar_add(s1f[:], wkt[:, K * H * F : K * H * F + H], 0.0)
        wm = wbuf.tile([P, K, H * F], bf16, tag="wm")
        for k in range(K):
            for h in range(H):
                nc.vector.tensor_scalar_mul(
                    wm[:, k, h * F : (h + 1) * F],
                    wkt[:, k * H * F + h * F : k * H * F + (h + 1) * F],
                    s1f[:, h : h + 1],
                )

        # ---- demod inputs: ssq[p, f] = sum_k sum_h wm^2.  bf16 throughout:
        # packed 16-bit reads run the DVE at 2x, and the denom matmul then
        # runs at bf16 LDWEIGHTS speed.  The ~0.3% rounding this adds to
        # denom is well inside the error budget. ----
        sq = wbuf.tile([P, K, H * F], bf16, tag="sq")
        nc.vector.tensor_mul(sq[:], wm[:], wm[:])
        acc = wbuf.tile([P, H * F], f32, tag="acc")
        nc.vector.tensor_add(acc[:], sq[:, 0], sq[:, 1])
        nc.vector.tensor_add(acc[:], acc[:], sq[:, 2])
        ssq = wbuf.tile([P, F], bf16, tag="ssq")
        nc.vector.tensor_add(ssq[:], acc[:, 0:F], acc[:, F : 2 * F])

        def emit_group(j, ft, i):
            """6 PSUM-accumulated matmuls for output tile (j, ft, i)."""
            ps = psum_pool.tile([P, WTILE], f32, tag="ps")
            order = [(k, h) for k in range(K) for h in range(H)]
            for n, (k, h) in enumerate(order):
                nc.tensor.matmul(
                    ps[:],
                    wm[:, k, h * F + ft * P : h * F + ft * P + P],
                    xm[:, h, j * WCHUNK + i * WTILE + k : j * WCHUNK + i * WTILE + k + WTILE],
                    start=(n == 0),
                    stop=(n == len(order) - 1),
                )
            return ps


        def emit_drain(st, ft, i, ps):
            # ft0 drains on DVE, ft1 on Scalar: both f-tiles of an i-step
            # drain in parallel, so PSUM slots free ~0.75us sooner and the
            # next chunk's first group never bubbles the LDWEIGHTS pipeline
            if ft == 0:
                nc.vector.tensor_scalar_mul(
                    st[:, i * WTILE : (i + 1) * WTILE], ps[:], denom[:, ft : ft + 1]
                )
            else:
                nc.scalar.mul(
                    st[:, i * WTILE : (i + 1) * WTILE], ps[:], denom[:, ft : ft + 1]
                )

        def emit_store(j, ft, st, lo=0, hi=WCHUNK):
            # stores ride the Activation HWDGE ring so their issue waits
            # never sit in front of load issues on the Sync ring
            out_rows = slice(ft * P, (ft + 1) * P)
            out_cols = slice(j * WCHUNK + lo, j * WCHUNK + hi)
            nc.scalar.dma_start(out[out_rows, out_cols], st[:, lo:hi])

        # ---- chunk 0: stash the first 4 groups, then the denom matmuls
        # (ssq lands ~when the PE finishes them), then drains (which gate
        # on denom in clean program order) ----
        sts = {
            ft: stage_pool.tile([P, WCHUNK], bf16, tag="stage", name=f"st0_{ft}")
            for ft in range(FT)
        }
        pend = []
        for i, ft in ((0, 0), (0, 1), (1, 0)):
            pend.append((ft, i, emit_group(0, ft, i)))
        denom = wbuf.tile([P, FT], f32, tag="denom")
        dp = psum_pool.tile([P, FT], f32, tag="ps", name="dp")
        for ft in range(FT):
            nc.tensor.matmul(
                dp[:, ft : ft + 1],
                ssq[:, ft * P : (ft + 1) * P],
                ones[:],
                start=True,
                stop=True,
            )
        nc.scalar.sqrt(denom[:], dp[:])
        nc.vector.reciprocal(denom[:], denom[:])
        for ft, i, ps in pend:
            emit_drain(sts[ft], ft, i, ps)
        ps = emit_group(0, 1, 1)
        emit_drain(sts[1], 1, 1, ps)
        for i in range(2, NI):
            for ft in range(FT):
                ps = emit_group(0, ft, i)
                emit_drain(sts[ft], ft, i, ps)
        for ft in range(FT):
            emit_store(0, ft, sts[ft])

        # ---- chunks 1..2: groups (i, ft)-interleaved; drains follow ----
        for j in range(1, NJ - 1):
            sts = {
                ft: stage_pool.tile([P, WCHUNK], bf16, tag="stage", name=f"st{j}_{ft}")
                for ft in range(FT)
            }
            for i in range(NI):
                for ft in range(FT):
                    ps = emit_group(j, ft, i)
                    emit_drain(sts[ft], ft, i, ps)
            for ft in range(FT):
                emit_store(j, ft, sts[ft])

        # ---- last chunk: ft-major so ft0's store overlaps ft1's compute,
        # and ft1's store is split so the very last transfer is small ----
        j = NJ - 1
        sts = {
            ft: stage_pool.tile([P, WCHUNK], bf16, tag="stage", name=f"st{j}_{ft}")
            for ft in range(FT)
        }
        for ft in range(FT):
            for i in range(NI):
                ps = emit_group(j, ft, i)
                if ft == 1 and i == NI - 1:
                    # final tile: drain halves on DVE + Scalar in parallel,
                    # store halves on Sync + Scalar, to shorten the tail
                    lo = i * WTILE
                    nc.vector.tensor_scalar_mul(
                        sts[1][:, lo : lo + WTILE // 2],
                        ps[:, 0 : WTILE // 2],
                        denom[:, 1:2],
                    )
                    nc.scalar.mul(
                        sts[1][:, lo + WTILE // 2 : lo + WTILE],
                        ps[:, WTILE // 2 : WTILE],
                        denom[:, 1:2],
                    )
                else:
                    emit_drain(sts[ft], ft, i, ps)
            if ft == 0:
                emit_store(j, 0, sts[0])
        emit_store(j, 1, sts[1], 0, 3 * WTILE)
        out_rows = slice(P, 2 * P)
        lo = j * WCHUNK + 3 * WTILE
        nc.sync.dma_start(
            out[out_rows, lo : lo + WTILE // 2],
            sts[1][:, 3 * WTILE : 3 * WTILE + WTILE // 2],
        )
        nc.scalar.dma_start(
            out[out_rows, lo + WTILE // 2 : lo + WTILE],
            sts[1][:, 3 * WTILE + WTILE // 2 : WCHUNK],
        )


def build_bass():
    nc = bass.Bass(name="conv1dmod")
    feat = nc.dram_tensor("feature", [C, W], mybir.dt.bfloat16, kind="ExternalInput")
    wk = nc.dram_tensor(
        "wk", [P, K * H * F + H], mybir.dt.bfloat16, kind="ExternalInput"
    )
    out = nc.dram_tensor("out", [F, W], mybir.dt.bfloat16, kind="ExternalOutput")
    with tile.TileContext(nc) as tc:
        _conv1dmod_body(tc, feat, wk, out)
    _split_sync_waits(nc)
    return nc


_NC_CACHE = None


def make_in_maps(feature, style, kernel):
    """Host-side prep: shard over batch, cast heavy tensors to bf16, pack
    kern as [p, (k h f)] (3 KB contiguous per partition) with the pre-folded
    modulate scale (1 + style) * coef appended as two bf16 columns."""
    bf16 = ml_dtypes.bfloat16
    feature = np.ascontiguousarray(feature).astype(bf16)
    kp = (
        np.asarray(kernel, dtype=np.float32)
        .reshape(K, P, H, F)
        .transpose(1, 0, 2, 3)
        .reshape(P, K * H * F)
    )
    s1 = ((np.asarray(style, dtype=np.float32) + 1.0) * COEF).reshape(B, P, H)
    wks = []
    for b in range(B):
        wk = np.concatenate([kp, s1[b]], axis=1).astype(bf16)
        wks.append(np.ascontiguousarray(wk))
    return [{"feature": feature[b], "wk": wks[b]} for b in range(B)]


def kernel(feature, style, kernel):
    """Full-input entry point: shard over batch across 8 cores, run, gather."""
    global _NC_CACHE
    from concourse.bass_utils import run_bass_kernel_spmd

    if _NC_CACHE is None:
        _NC_CACHE = build_bass()
    nc = _NC_CACHE

    in_maps = make_in_maps(feature, style, kernel)
    res = run_bass_kernel_spmd(nc, in_maps, core_ids=list(range(B)))
    return np.stack(
        [r["out"].astype(np.float32) for r in res.results], axis=0
    )


# revision 22
# speedup vs baseline: 1.1825x; 1.1825x over previous
"""Trainium2 Bass kernel for modulated conv1d (StyleGAN-style Conv1DMod).

Reference computation (per batch sample b):
  wm[k,c,f]  = kern[k,c,f] * coef * (style[b,c] + 1)        (modulate)
  denom[f]   = rsqrt(sum_{k,c} wm[k,c,f]^2)                 (demodulate)
  out[b,f,w] = denom[f] * sum_{k,c} wm[k,c,f] * feat[b,c,w+k-1]   (SAME conv)

Sharding: data-parallel over batch B=8 -> one sample per NeuronCore.

Schedule notes (final):
 - All heavy I/O is bf16: feature and kern are cast to bf16 on the host
   (the PE rounds products to bf16 anyway; rel err ~4e-3 vs the 2e-2
   gate), and the output is stored bf16 and widened to f32 on the host.
   HBM traffic drops 16.8 MB -> 8.9 MB per core, far below the ~41.5 us
   PE floor (192 matmuls x 216 ns), so the Tensor engine is the only
   roofline.  kern and the host-prefolded (1+style)*coef vector are
   packed into one [128, 1538] bf16 tensor with 3 KB contiguous
   per-partition runs.
 - One resident x mega-tile [128, 2, 8194] holds the whole padded
   feature row (c = 2p + h); chunk boundaries need no halo duplication
   and conv taps are plain column slices.
 - DMA completion semaphores lag the last byte by ~3us and serialize
   per HWDGE ring (~+2-3us per DMA), so early pieces are spread across
   all three paths: wk + P1b on Sync, P0 on Activation, the i=1 half
   of chunk 0 on the GpSimd SWDGE, and the remaining chunks on
   Activation behind the act-table load.
 - 70 dep-free N=64 dummy matmuls keep the PE busy from barrier-exit to
   wm-ready (~4.3us): the HAM un-throttles to 2.4 GHz during the bridge
   and never re-throttles, so the real stream runs warm end to end.
 - demod runs bf16 on the DVE; the two denom matmuls sit three conv
   groups into the stream (ssq lands just before the PE does) so PSUM
   drains (DVE, demod scale fused, bf16 out) start early.
 - The final tile's drain and store are split across DVE+Scalar and
   Sync+Scalar so the tail is issue/receipt-latency bound only.
"""

import numpy as np
import ml_dtypes

import concourse.bass as bass
import concourse.mybir as mybir
import concourse.tile as tile

B, C, W, K, F = 8, 256, 8192, 3, 256
COEF = 1.0 / float(np.sqrt(K * C))

P = 128
H = 2  # contraction groups: c = 2*p + h
FT = F // P  # 2 output-partition tiles
WCHUNK = 2048
NJ = W // WCHUNK  # 4 chunks
WTILE = 512  # matmul moving-operand width (PSUM bank = 512 f32)
NI = WCHUNK // WTILE  # 4 w-tiles per chunk
XW = W + 2  # mega tile cols: col t holds feat[t-1], cols 0 and W+1 are zero

MAX_WAITS = 1  # walrus codegen in this container rejects >1 sync wait per inst


def _split_sync_waits(nc, limit=MAX_WAITS):
    """Move excess sem-waits onto NoOps inserted before the offending
    instruction (same engine, program order preserved)."""
    uid = 0
    for fn in nc.m.functions:
        for bb in fn.blocks:
            insts = bb.instructions
            changed = False
            newlist = []
            for ins in insts:
                si = ins.sync_info
                if si is not None and len(si.on_wait) > limit:
                    waits = list(si.on_wait)
                    keep = waits[-limit:]
                    excess = waits[:-limit]
                    for k in range(0, len(excess), limit):
                        nop = mybir.InstNoOp(name=f"waitsplit-{uid}", ins=[], outs=[])
                        uid += 1
                        nop.engine = ins.engine
                        nop.sync_info = mybir.SyncInfo(
                            on_wait=excess[k : k + limit], on_update=[]
                        )
                        newlist.append(nop)
                    ins.sync_info = mybir.SyncInfo(
                        on_wait=keep, on_update=list(si.on_update)
                    )
                    changed = True
                newlist.append(ins)
            if changed:
                bb.instructions = newlist


def _conv1dmod_body(tc, feat, wk, out):
    nc = tc.nc
    f32 = mybir.dt.float32
    bf16 = mybir.dt.bfloat16
    add = mybir.AluOpType.add
    mult = mybir.AluOpType.mult

    featr = feat.rearrange("(p h) w -> p h w", h=H)

    with (
        tc.tile_pool(name="wbuf", bufs=1) as wbuf,
        tc.tile_pool(name="xbuf", bufs=1) as xbuf,
        tc.tile_pool(name="stage", bufs=4) as stage_pool,
        tc.tile_pool(name="psum", bufs=8, space="PSUM") as psum_pool,
    ):
        # ---- warm-up operand first on DVE: it gates the PE dummies ----
        wu_w = wbuf.tile([P, P], bf16, tag="wu_w")
        nc.vector.memset(wu_w[:], 0.0)
        warm = wbuf.tile([P, 1], f32, tag="warm")
        nc.vector.memset(warm[:], 1.0)
        ones = wbuf.tile([P, 1], bf16, tag="ones")
        nc.vector.memset(ones[:], 1.0)

        # ---- x mega-tile (bf16) + zero halo columns ----
        xm = xbuf.tile([P, H, XW], bf16, tag="xm")
        for h in range(H):
            nc.vector.memset(xm[:, h, 0:1], 0.0)
            nc.vector.memset(xm[:, h, XW - 1 : XW], 0.0)

        # ---- head DMAs.  wk = host-packed [kern | (1+style)*coef] with 3 KB
        # contiguous per-partition runs.  DMA completion semaphores lag the
        # last byte by 1-4us depending on concurrent HBM load, so only wk,
        # P0 and P1 move in the first window; P2-P4 issue on the Activation
        # ring behind the act-table load, keeping HBM quiet while the wk
        # receipt (which gates modulates -> the whole PE stream) returns.
        wkt = wbuf.tile([P, K * H * F + H], bf16, tag="wkt")
        nc.sync.dma_start(wkt[:], wk[:, :])
        pieces = [0, 518, 2048, 4096, 6144, 8192]
        a, b = pieces[0], pieces[1]
        nc.scalar.dma_start(xm[:, :, a + 1 : b + 1], featr[:, :, a:b])
        # Completion receipts serialize per HWDGE ring (~+2-3us per DMA), so
        # a ring's second semaphore can't beat the PE's i=1 demand (~2.6us
        # into the stream).  The i=1 half of P1 rides the independent GpSimd
        # SWDGE path instead; the rest of P1 queues behind wk on Sync.
        nc.gpsimd.dma_start(xm[:, :, 519:1032], featr[:, :, 518:1031])
        nc.sync.dma_start(xm[:, :, 1032:2049], featr[:, :, 1031:2048])

        # warm the Scalar activation table (Sqrt) off the critical path
        warm2 = wbuf.tile([P, 1], f32, tag="warm2")
        nc.scalar.sqrt(warm2[:], warm[:])

        for pc in range(2, len(pieces) - 1):
            a, b = pieces[pc], pieces[pc + 1]
            nc.scalar.dma_start(xm[:, :, a + 1 : b + 1], featr[:, :, a:b])

        # ---- PE p-state warm-up: dep-free dummy matmuls keep the Tensor
        # engine busy (HAM un-throttles to 2.4 GHz after ~3.4us of activity)
        # and bridge all the way to wm-ready (~4.3us), so the real stream
        # starts warm with no PE-idle window for the HAM to re-throttle in.
        # N=64 keeps the bridge quantization (and any overshoot) tiny.
        wu_ps = psum_pool.tile([P, P], f32, tag="ps")
        for _ in range(70):
            nc.tensor.matmul(
                wu_ps[:, 0:64], wu_w[:], wu_w[:, 0:64], start=True, stop=True
            )

        # ---- modulate (bf16 weights) on DVE; scale vector comes in
        # pre-folded as (1 + style) * coef from the host, bf16, packed as
        # the last two wk columns ----
        s1f = wbuf.tile([P, H], f32, tag="s1f")
        nc.vector.tensor_scalar_add(s1f[:], wkt[:, K * H * F : K * H * F + H], 0.0)
        wm = wbuf.tile([P, K, H * F], bf16, tag="wm")
        for k in range(K):
            for h in range(H):
                nc.vector.tensor_scalar_mul(
                    wm[:, k, h * F : (h + 1) * F],
                    wkt[:, k * H * F + h * F : k * H * F + (h + 1) * F],
                    s1f[:, h : h + 1],
                )

        # ---- demod inputs: ssq[p, f] = sum_k sum_h wm^2.  bf16 throughout:
        # packed 16-bit reads run the DVE at 2x, and the denom matmul then
        # runs at bf16 LDWEIGHTS speed.  The ~0.3% rounding this adds to
        # denom is well inside the error budget. ----
        sq = wbuf.tile([P, K, H * F], bf16, tag="sq")
        nc.vector.tensor_mul(sq[:], wm[:], wm[:])
        acc = wbuf.tile([P, H * F], f32, tag="acc")
        nc.vector.tensor_add(acc[:], sq[:, 0], sq[:, 1])
        nc.vector.tensor_add(acc[:], acc[:], sq[:, 2])
        ssq = wbuf.tile([P, F], bf16, tag="ssq")
        nc.vector.tensor_add(ssq[:], acc[:, 0:F], acc[:, F : 2 * F])

        def emit_group(j, ft, i):
            """6 PSUM-accumulated matmuls for output tile (j, ft, i)."""
            ps = psum_pool.tile([P, WTILE], f32, tag="ps")
            order = [(k, h) for k in range(K) for h in range(H)]
            for n, (k, h) in enumerate(order):
                nc.tensor.matmul(
                    ps[:],
                    wm[:, k, h * F + ft * P : h * F + ft * P + P],
                    xm[:, h, j * WCHUNK + i * WTILE + k : j * WCHUNK + i * WTILE + k + WTILE],
                    start=(n == 0),
                    stop=(n == len(order) - 1),
                )
            return ps


        def emit_drain(st, ft, i, ps):
            nc.vector.tensor_scalar_mul(
                st[:, i * WTILE : (i + 1) * WTILE], ps[:], denom[:, ft : ft + 1]
            )

        def emit_store(j, ft, st, lo=0, hi=WCHUNK):
            # stores ride the Activation HWDGE ring so their issue waits
            # never sit in front of load issues on the Sync ring
            out_rows = slice(ft * P, (ft + 1) * P)
            out_cols = slice(j * WCHUNK + lo, j * WCHUNK + hi)
            nc.scalar.dma_start(out[out_rows, out_cols], st[:, lo:hi])

        # ---- chunk 0: stash the first 4 groups, then the denom matmuls
        # (ssq lands ~when the PE finishes them), then drains (which gate
        # on denom in clean program order) ----
        sts = {
            ft: stage_pool.tile([P, WCHUNK], bf16, tag="stage", name=f"st0_{ft}")
            for ft in range(FT)
        }
        pend = []
        for i, ft in ((0, 0), (0, 1), (1, 0)):
            pend.append((ft, i, emit_group(0, ft, i)))
        denom = wbuf.tile([P, FT], f32, tag="denom")
        dp = psum_pool.tile([P, FT], f32, tag="ps", name="dp")
        for ft in range(FT):
            nc.tensor.matmul(
                dp[:, ft : ft + 1],
                ssq[:, ft * P : (ft + 1) * P],
                ones[:],
                start=True,
                stop=True,
            )
        nc.scalar.sqrt(denom[:], dp[:])
        nc.vector.reciprocal(denom[:], denom[:])
        for ft, i, ps in pend:
            emit_drain(sts[ft], ft, i, ps)
        ps = emit_group(0, 1, 1)
        emit_drain(sts[1], 1, 1, ps)
        for i in range(2, NI):
            for ft in range(FT):
                ps = emit_group(0, ft, i)
                emit_drain(sts[ft], ft, i, ps)
        for ft in range(FT):
            emit_store(0, ft, sts[ft])

        # ---- chunks 1..2: groups (i, ft)-interleaved; drains follow ----
        for j in range(1, NJ - 1):
            sts = {
                ft: stage_pool.tile([P, WCHUNK], bf16, tag="stage", name=f"st{j}_{ft}")
                for ft in range(FT)
            }
            for i in range(NI):
                for ft in range(FT):
                    ps = emit_group(j, ft, i)
                    emit_drain(sts[ft], ft, i, ps)
            for ft in range(FT):
                emit_store(j, ft, sts[ft])

        # ---- last chunk: ft-major so ft0's store overlaps ft1's compute,
        # and ft1's store is split so the very last transfer is small ----
        j = NJ - 1
        sts = {
            ft: stage_pool.tile([P, WCHUNK], bf16, tag="stage", name=f"st{j}_{ft}")
            for ft in range(FT)
        }
        for ft in range(FT):
            for i in range(NI):
                ps = emit_group(j, ft, i)
                if ft == 1 and i == NI - 1:
                    # final tile: drain halves on DVE + Scalar in parallel,
                    # store halves on Sync + Scalar, to shorten the tail
                    lo = i * WTILE
                    nc.vector.tensor_scalar_mul(
                        sts[1][:, lo : lo + WTILE // 2],
                        ps[:, 0 : WTILE // 2],
                        denom[:, 1:2],
                    )
                    nc.scalar.mul(
                        sts[1][:, lo + WTILE // 2 : lo + WTILE],
                        ps[:, WTILE // 2 : WTILE],
                        denom[:, 1:2],
                    )
                else:
                    emit_drain(sts[ft], ft, i, ps)
            if ft == 0:
                emit_store(j, 0, sts[0])
        emit_store(j, 1, sts[1], 0, 3 * WTILE)
        out_rows = slice(P, 2 * P)
        lo = j * WCHUNK + 3 * WTILE
        nc.sync.dma_start(
            out[out_rows, lo : lo + WTILE // 2],
            sts[1][:, 3 * WTILE : 3 * WTILE + WTILE // 2],
        )
        nc.scalar.dma_start(
            out[out_rows, lo + WTILE // 2 : lo + WTILE],
            sts[1][:, 3 * WTILE + WTILE // 2 : WCHUNK],
        )


def build_bass():
    nc = bass.Bass(name="conv1dmod")
    feat = nc.dram_tensor("feature", [C, W], mybir.dt.bfloat16, kind="ExternalInput")
    wk = nc.dram_tensor(
        "wk", [P, K * H * F + H], mybir.dt.bfloat16, kind="ExternalInput"
    )
    out = nc.dram_tensor("out", [F, W], mybir.dt.bfloat16, kind="ExternalOutput")
    with tile.TileContext(nc) as tc:
        _conv1dmod_body(tc, feat, wk, out)
    _split_sync_waits(nc)
    return nc


_NC_CACHE = None


def make_in_maps(feature, style, kernel):
    """Host-side prep: shard over batch, cast heavy tensors to bf16, pack
    kern as [p, (k h f)] (3 KB contiguous per partition) with the pre-folded
    modulate scale (1 + style) * coef appended as two bf16 columns."""
    bf16 = ml_dtypes.bfloat16
    feature = np.ascontiguousarray(feature).astype(bf16)
    kp = (
        np.asarray(kernel, dtype=np.float32)
        .reshape(K, P, H, F)
        .transpose(1, 0, 2, 3)
        .reshape(P, K * H * F)
    )
    s1 = ((np.asarray(style, dtype=np.float32) + 1.0) * COEF).reshape(B, P, H)
    wks = []
    for b in range(B):
        wk = np.concatenate([kp, s1[b]], axis=1).astype(bf16)
        wks.append(np.ascontiguousarray(wk))
    return [{"feature": feature[b], "wk": wks[b]} for b in range(B)]


def kernel(feature, style, kernel):
    """Full-input entry point: shard over batch across 8 cores, run, gather."""
    global _NC_CACHE
    from concourse.bass_utils import run_bass_kernel_spmd

    if _NC_CACHE is None:
        _NC_CACHE = build_bass()
    nc = _NC_CACHE

    in_maps = make_in_maps(feature, style, kernel)
    res = run_bass_kernel_spmd(nc, in_maps, core_ids=list(range(B)))
    return np.stack(
        [r["out"].astype(np.float32) for r in res.results], axis=0
    )


# revision 23
# speedup vs baseline: 1.1873x; 1.0040x over previous
"""Trainium2 Bass kernel for modulated conv1d (StyleGAN-style Conv1DMod).

Reference computation (per batch sample b):
  wm[k,c,f]  = kern[k,c,f] * coef * (style[b,c] + 1)        (modulate)
  denom[f]   = rsqrt(sum_{k,c} wm[k,c,f]^2)                 (demodulate)
  out[b,f,w] = denom[f] * sum_{k,c} wm[k,c,f] * feat[b,c,w+k-1]   (SAME conv)

Sharding: data-parallel over batch B=8 -> one sample per NeuronCore.

Schedule notes (final):
 - All heavy I/O is bf16: feature and kern are cast to bf16 on the host
   (the PE rounds products to bf16 anyway; rel err ~4e-3 vs the 2e-2
   gate), and the output is stored bf16 and widened to f32 on the host.
   HBM traffic drops 16.8 MB -> 8.9 MB per core, far below the ~41.5 us
   PE floor (192 matmuls x 216 ns), so the Tensor engine is the only
   roofline.  kern and the host-prefolded (1+style)*coef vector are
   packed into one [128, 1538] bf16 tensor with 3 KB contiguous
   per-partition runs.
 - One resident x mega-tile [128, 2, 8194] holds the whole padded
   feature row (c = 2p + h); chunk boundaries need no halo duplication
   and conv taps are plain column slices.
 - DMA completion semaphores lag the last byte by ~3us and serialize
   per HWDGE ring (~+2-3us per DMA), so early pieces are spread across
   all three paths: wk + P1b on Sync, P0 on Activation, the i=1 half
   of chunk 0 on the GpSimd SWDGE, and the remaining chunks on
   Activation behind the act-table load.
 - 70 dep-free N=64 dummy matmuls keep the PE busy from barrier-exit to
   wm-ready (~4.3us): the HAM un-throttles to 2.4 GHz during the bridge
   and never re-throttles, so the real stream runs warm end to end.
 - demod runs bf16 on the DVE; the two denom matmuls sit three conv
   groups into the stream (ssq lands just before the PE does) so PSUM
   drains (DVE, demod scale fused, bf16 out) start early.
 - The final tile's drain and store are split across DVE+Scalar and
   Sync+Scalar so the tail is issue/receipt-latency bound only.
"""

import numpy as np
import ml_dtypes

import concourse.bass as bass
import concourse.mybir as mybir
import concourse.tile as tile

B, C, W, K, F = 8, 256, 8192, 3, 256
COEF = 1.0 / float(np.sqrt(K * C))

P = 128
H = 2  # contraction groups: c = 2*p + h
FT = F // P  # 2 output-partition tiles
WCHUNK = 2048
NJ = W // WCHUNK  # 4 chunks
WTILE = 512  # matmul moving-operand width (PSUM bank = 512 f32)
NI = WCHUNK // WTILE  # 4 w-tiles per chunk
XW = W + 2  # mega tile cols: col t holds feat[t-1], cols 0 and W+1 are zero

MAX_WAITS = 1  # walrus codegen in this container rejects >1 sync wait per inst


def _split_sync_waits(nc, limit=MAX_WAITS):
    """Move excess sem-waits onto NoOps inserted before the offending
    instruction (same engine, program order preserved)."""
    uid = 0
    for fn in nc.m.functions:
        for bb in fn.blocks:
            insts = bb.instructions
            changed = False
            newlist = []
            for ins in insts:
                si = ins.sync_info
                if si is not None and len(si.on_wait) > limit:
                    waits = list(si.on_wait)
                    keep = waits[-limit:]
                    excess = waits[:-limit]
                    for k in range(0, len(excess), limit):
                        nop = mybir.InstNoOp(name=f"waitsplit-{uid}", ins=[], outs=[])
                        uid += 1
                        nop.engine = ins.engine
                        nop.sync_info = mybir.SyncInfo(
                            on_wait=excess[k : k + limit], on_update=[]
                        )
                        newlist.append(nop)
                    ins.sync_info = mybir.SyncInfo(
                        on_wait=keep, on_update=list(si.on_update)
                    )
                    changed = True
                newlist.append(ins)
            if changed:
                bb.instructions = newlist


def _conv1dmod_body(tc, feat, wk, out):
    nc = tc.nc
    f32 = mybir.dt.float32
    bf16 = mybir.dt.bfloat16
    add = mybir.AluOpType.add
    mult = mybir.AluOpType.mult

    featr = feat.rearrange("(p h) w -> p h w", h=H)

    with (
        tc.tile_pool(name="wbuf", bufs=1) as wbuf,
        tc.tile_pool(name="xbuf", bufs=1) as xbuf,
        tc.tile_pool(name="stage", bufs=4) as stage_pool,
        tc.tile_pool(name="psum", bufs=8, space="PSUM") as psum_pool,
    ):
        # ---- warm-up operand first on DVE: it gates the PE dummies ----
        wu_w = wbuf.tile([P, P], bf16, tag="wu_w")
        nc.vector.memset(wu_w[:], 0.0)
        warm = wbuf.tile([P, 1], f32, tag="warm")
        nc.vector.memset(warm[:], 1.0)
        ones = wbuf.tile([P, 1], bf16, tag="ones")
        nc.vector.memset(ones[:], 1.0)

        # ---- x mega-tile (bf16) + zero halo columns ----
        xm = xbuf.tile([P, H, XW], bf16, tag="xm")
        for h in range(H):
            nc.vector.memset(xm[:, h, 0:1], 0.0)
            nc.vector.memset(xm[:, h, XW - 1 : XW], 0.0)

        # ---- head DMAs.  wk = host-packed [kern | (1+style)*coef] with 3 KB
        # contiguous per-partition runs.  DMA completion semaphores lag the
        # last byte by 1-4us depending on concurrent HBM load, so only wk,
        # P0 and P1 move in the first window; P2-P4 issue on the Activation
        # ring behind the act-table load, keeping HBM quiet while the wk
        # receipt (which gates modulates -> the whole PE stream) returns.
        wkt = wbuf.tile([P, K * H * F + H], bf16, tag="wkt")
        nc.sync.dma_start(wkt[:], wk[:, :])
        pieces = [0, 518, 2048, 4096, 6144, 8192]
        a, b = pieces[0], pieces[1]
        nc.scalar.dma_start(xm[:, :, a + 1 : b + 1], featr[:, :, a:b])
        # Completion receipts serialize per HWDGE ring (~+2-3us per DMA), so
        # a ring's second semaphore can't beat the PE's i=1 demand (~2.6us
        # into the stream).  The i=1 half of P1 rides the independent GpSimd
        # SWDGE path instead; the rest of P1 queues behind wk on Sync.
        nc.gpsimd.dma_start(xm[:, :, 519:1032], featr[:, :, 518:1031])
        nc.sync.dma_start(xm[:, :, 1032:2049], featr[:, :, 1031:2048])

        # warm the Scalar activation table (Sqrt) off the critical path
        warm2 = wbuf.tile([P, 1], f32, tag="warm2")
        nc.scalar.sqrt(warm2[:], warm[:])

        for pc in range(2, len(pieces) - 1):
            a, b = pieces[pc], pieces[pc + 1]
            nc.scalar.dma_start(xm[:, :, a + 1 : b + 1], featr[:, :, a:b])

        # ---- PE p-state warm-up: dep-free dummy matmuls keep the Tensor
        # engine busy (HAM un-throttles to 2.4 GHz after ~3.4us of activity)
        # and bridge all the way to wm-ready (~4.3us), so the real stream
        # starts warm with no PE-idle window for the HAM to re-throttle in.
        # N=64 keeps the bridge quantization (and any overshoot) tiny.
        wu_ps = psum_pool.tile([P, P], f32, tag="ps")
        for _ in range(70):
            nc.tensor.matmul(
                wu_ps[:, 0:64], wu_w[:], wu_w[:, 0:64], start=True, stop=True
            )

        # ---- modulate (bf16 weights) on DVE; scale vector comes in
        # pre-folded as (1 + style) * coef from the host, bf16, packed as
        # the last two wk columns ----
        s1f = wbuf.tile([P, H], f32, tag="s1f")
        nc.vector.tensor_scalar_add(s1f[:], wkt[:, K * H * F : K * H * F + H], 0.0)
        wm = wbuf.tile([P, K, H * F], bf16, tag="wm")
        # modulate in [128,128] pieces, ft0 first: the piece supply order
        # then exactly tracks the first conv groups' stationary-weight
        # demand order, so the stream start never chases a late modulate
        for ft in range(FT):
            for k in range(K):
                for h in range(H):
                    lo = h * F + ft * P
                    nc.vector.tensor_scalar_mul(
                        wm[:, k, lo : lo + P],
                        wkt[:, k * H * F + lo : k * H * F + lo + P],
                        s1f[:, h : h + 1],
                    )

        # ---- demod inputs: ssq[p, f] = sum_k sum_h wm^2.  bf16 throughout:
        # packed 16-bit reads run the DVE at 2x, and the denom matmul then
        # runs at bf16 LDWEIGHTS speed.  The ~0.3% rounding this adds to
        # denom is well inside the error budget. ----
        sq = wbuf.tile([P, K, H * F], bf16, tag="sq")
        nc.vector.tensor_mul(sq[:], wm[:], wm[:])
        acc = wbuf.tile([P, H * F], f32, tag="acc")
        nc.vector.tensor_add(acc[:], sq[:, 0], sq[:, 1])
        nc.vector.tensor_add(acc[:], acc[:], sq[:, 2])
        ssq = wbuf.tile([P, F], bf16, tag="ssq")
        nc.vector.tensor_add(ssq[:], acc[:, 0:F], acc[:, F : 2 * F])

        def emit_group(j, ft, i):
            """6 PSUM-accumulated matmuls for output tile (j, ft, i)."""
            ps = psum_pool.tile([P, WTILE], f32, tag="ps")
            order = [(k, h) for k in range(K) for h in range(H)]
            for n, (k, h) in enumerate(order):
                nc.tensor.matmul(
                    ps[:],
                    wm[:, k, h * F + ft * P : h * F + ft * P + P],
                    xm[:, h, j * WCHUNK + i * WTILE + k : j * WCHUNK + i * WTILE + k + WTILE],
                    start=(n == 0),
                    stop=(n == len(order) - 1),
                )
            return ps


        def emit_drain(st, ft, i, ps):
            nc.vector.tensor_scalar_mul(
                st[:, i * WTILE : (i + 1) * WTILE], ps[:], denom[:, ft : ft + 1]
            )

        def emit_store(j, ft, st, lo=0, hi=WCHUNK):
            # stores ride the Activation HWDGE ring so their issue waits
            # never sit in front of load issues on the Sync ring
            out_rows = slice(ft * P, (ft + 1) * P)
            out_cols = slice(j * WCHUNK + lo, j * WCHUNK + hi)
            nc.scalar.dma_start(out[out_rows, out_cols], st[:, lo:hi])

        # ---- chunk 0: stash the first 4 groups, then the denom matmuls
        # (ssq lands ~when the PE finishes them), then drains (which gate
        # on denom in clean program order) ----
        sts = {
            ft: stage_pool.tile([P, WCHUNK], bf16, tag="stage", name=f"st0_{ft}")
            for ft in range(FT)
        }
        pend = []
        for i, ft in ((0, 0), (0, 1), (1, 0)):
            pend.append((ft, i, emit_group(0, ft, i)))
        denom = wbuf.tile([P, FT], f32, tag="denom")
        dp = psum_pool.tile([P, FT], f32, tag="ps", name="dp")
        for ft in range(FT):
            nc.tensor.matmul(
                dp[:, ft : ft + 1],
                ssq[:, ft * P : (ft + 1) * P],
                ones[:],
                start=True,
                stop=True,
            )
        nc.scalar.sqrt(denom[:], dp[:])
        nc.vector.reciprocal(denom[:], denom[:])
        for ft, i, ps in pend:
            emit_drain(sts[ft], ft, i, ps)
        ps = emit_group(0, 1, 1)
        emit_drain(sts[1], 1, 1, ps)
        for i in range(2, NI):
            for ft in range(FT):
                ps = emit_group(0, ft, i)
                emit_drain(sts[ft], ft, i, ps)
        for ft in range(FT):
            emit_store(0, ft, sts[ft])

        # ---- chunks 1..2: groups (i, ft)-interleaved; drains follow ----
        for j in range(1, NJ - 1):
            sts = {
                ft: stage_pool.tile([P, WCHUNK], bf16, tag="stage", name=f"st{j}_{ft}")
                for ft in range(FT)
            }
            for i in range(NI):
                for ft in range(FT):
                    ps = emit_group(j, ft, i)
                    emit_drain(sts[ft], ft, i, ps)
            for ft in range(FT):
                emit_store(j, ft, sts[ft])

        # ---- last chunk: ft-major so ft0's store overlaps ft1's compute,
        # and ft1's store is split so the very last transfer is small ----
        j = NJ - 1
        sts = {
            ft: stage_pool.tile([P, WCHUNK], bf16, tag="stage", name=f"st{j}_{ft}")
            for ft in range(FT)
        }
        for ft in range(FT):
            for i in range(NI):
                ps = emit_group(j, ft, i)
                if ft == 1 and i == NI - 1:
                    # final tile: drain halves on DVE + Scalar in parallel,
                    # store halves on Sync + Scalar, to shorten the tail
                    lo = i * WTILE
                    nc.vector.tensor_scalar_mul(
                        sts[1][:, lo : lo + WTILE // 2],
                        ps[:, 0 : WTILE // 2],
                        denom[:, 1:2],
                    )
                    nc.scalar.mul(
                        sts[1][:, lo + WTILE // 2 : lo + WTILE],
                        ps[:, WTILE // 2 : WTILE],
                        denom[:, 1:2],
                    )
                else:
                    emit_drain(sts[ft], ft, i, ps)
            if ft == 0:
                emit_store(j, 0, sts[0])
        emit_store(j, 1, sts[1], 0, 3 * WTILE)
        out_rows = slice(P, 2 * P)
        lo = j * WCHUNK + 3 * WTILE
        nc.sync.dma_start(
            out[out_rows, lo : lo + WTILE // 2],
            sts[1][:, 3 * WTILE : 3 * WTILE + WTILE // 2],
        )
        nc.scalar.dma_start(
            out[out_rows, lo + WTILE // 2 : lo + WTILE],
            sts[1][:, 3 * WTILE + WTILE // 2 : WCHUNK],
        )


def build_bass():
    nc = bass.Bass(name="conv1dmod")
    feat = nc.dram_tensor("feature", [C, W], mybir.dt.bfloat16, kind="ExternalInput")
    wk = nc.dram_tensor(
        "wk", [P, K * H * F + H], mybir.dt.bfloat16, kind="ExternalInput"
    )
    out = nc.dram_tensor("out", [F, W], mybir.dt.bfloat16, kind="ExternalOutput")
    with tile.TileContext(nc) as tc:
        _conv1dmod_body(tc, feat, wk, out)
    _split_sync_waits(nc)
    return nc


_NC_CACHE = None


def make_in_maps(feature, style, kernel):
    """Host-side prep: shard over batch, cast heavy tensors to bf16, pack
    kern as [p, (k h f)] (3 KB contiguous per partition) with the pre-folded
    modulate scale (1 + style) * coef appended as two bf16 columns."""
    bf16 = ml_dtypes.bfloat16
    feature = np.ascontiguousarray(feature).astype(bf16)
    kp = (
        np.asarray(kernel, dtype=np.float32)
        .reshape(K, P, H, F)
        .transpose(1, 0, 2, 3)
        .reshape(P, K * H * F)
    )
    s1 = ((np.asarray(style, dtype=np.float32) + 1.0) * COEF).reshape(B, P, H)
    wks = []
    for b in range(B):
        wk = np.concatenate([kp, s1[b]], axis=1).astype(bf16)
        wks.append(np.ascontiguousarray(wk))
    return [{"feature": feature[b], "wk": wks[b]} for b in range(B)]


def kernel(feature, style, kernel):
    """Full-input entry point: shard over batch across 8 cores, run, gather."""
    global _NC_CACHE
    from concourse.bass_utils import run_bass_kernel_spmd

    if _NC_CACHE is None:
        _NC_CACHE = build_bass()
    nc = _NC_CACHE

    in_maps = make_in_maps(feature, style, kernel)
    res = run_bass_kernel_spmd(nc, in_maps, core_ids=list(range(B)))
    return np.stack(
        [r["out"].astype(np.float32) for r in res.results], axis=0
    )
